# revision 42
# baseline (speedup 1.0000x reference)
"""Trainium2 Bass kernel: ViT-style global attention with decomposed
relative position bias (B=8, 32x32 tokens, dim 768, 12 heads, hd 64).

Sharding: data-parallel over batch B=8 -> one image per NeuronCore,
weights replicated, no collectives.

v2 design (fp8 + phase restructure), per core:
  * QKV projection in fp8e4 DoubleRow, 3-pass residual form:
    host ships x and 32*W as fp8 (hi, lo=residual) pairs; kernel computes
    x_hi@W_hi + x_lo@W_hi + x_hi@W_lo (lo@lo dropped, ~bf16 accuracy) at
    0.5 cycles/row -> 2304 cycles per 128x512 output tile vs 3072 fp32r.
    Drains scale by 1/32 and add the bias (ACT for q -> QALL8 fp8,
    DVE for k -> KALL8 fp8).
  * Attention scores in ONE fp8 DoubleRow matmul per (head, kblock, qh):
    contraction pair = (64 q-features, 64 rel-bias rows).  QALL8/KALL8
    hold per-head [64p, 2, N] slices (even heads partitions 0:64, odd
    64:128); slice1 carries 8*rel values (q side) and 0/1 indicator rows
    (k side).  exp on ACT applies the 1/8 softmax scale -> bf16 P.
  * rel-pos matmuls read q directly from QALL8 (fp8, no staging copy);
    block-diagonal bd tables are fp8 with columns placed so copies into
    QALL8 slice1 stay partition-aligned (ACT even/h, Pool odd/w).
  * V per head-pair: 9 fp8 DoubleRow matmuls per 2-token-blocks + bf16
    K=1 ones-row matmul rides the 32x v-bias; Pool drains into the
    parity-coded V'' = [V|1] / [0,1,0,V] bf16 layout (32x scaled; the
    proj drain divides by 32).  Denominator rides the PV matmul.
  * Phase order: q chunks -> rel-pos -> k chunk 0 -> attention heads
    0..11 with k chunks 1..5 and V pairs 1..5 interleaved into PE slack
    (the attention phase is ACT(exp)-bound); proj fp32r at the tail.
"""

import os
import numpy as np

import concourse.bacc as bacc
import concourse.bass as bass
import concourse.tile as tile
from concourse import mybir
from concourse import bass_utils

B, H, W, DIM = 8, 32, 32, 768
HEADS, HD = 12, 64
N = H * W  # 1024
NCORES = 8
SCALE = HD ** -0.5
WS = 32.0           # fp8 weight pre-scale
F32 = mybir.dt.float32
F32R = mybir.dt.float32r
BF16 = mybir.dt.bfloat16
F8 = mybir.dt.float8e4
EXP = mybir.ActivationFunctionType.Exp
IDN = mybir.ActivationFunctionType.Identity
ADD = mybir.AluOpType.add
MULT = mybir.AluOpType.mult
DR = mybir.MatmulPerfMode.DoubleRow

NC = DIM // 128      # 6 feature chunks
NPAIR = NC // 2      # 3 contraction chunk-pairs for DoubleRow
NKB = N // 128       # 8 key blocks
NQH = N // 512       # 2 query halves
NPR = HEADS // 2     # 6 head pairs
VW = 65 + 128        # even (V|1) + odd (0,1,0,V) stationary widths

_CACHE = {}

K_WARM = int(os.environ.get("K_WARM", "12"))
K_PT = int(os.environ.get("K_PT", "10"))
K_PS1 = int(os.environ.get("K_PS1", "4"))
K_PS2 = int(os.environ.get("K_PS2", "2"))
K_PSS = int(os.environ.get("K_PSS", "2"))
K_SM = int(os.environ.get("K_SM", "4"))
K_PS4 = int(os.environ.get("K_PS4", "4"))


def build_nc():
    nc = bacc.Bacc("TRN2", target_bir_lowering=False, debug=False)

    x8hi_d = nc.dram_tensor("x8hi", (128, NC, N), F8, kind="ExternalInput").ap()
    x8lo_d = nc.dram_tensor("x8lo", (128, NC, N), F8, kind="ExternalInput").ap()
    w8hi_d = nc.dram_tensor("w8hi", (128, NPAIR, 2, 3 * DIM), F8,
                            kind="ExternalInput").ap()
    w8lo_d = nc.dram_tensor("w8lo", (128, NPAIR, 2, 3 * DIM), F8,
                            kind="ExternalInput").ap()
    qkvb_d = nc.dram_tensor("qkvb18", (128, 18), F32, kind="ExternalInput").ap()
    bd8_d = nc.dram_tensor("bd8", (128, 2, 32, 128), F8,
                           kind="ExternalInput").ap()
    kc8_d = nc.dram_tensor("kc8rep", (128, NPR, N), F8,
                           kind="ExternalInput").ap()
    vbrow_d = nc.dram_tensor("vbrow32", (1, DIM), BF16, kind="ExternalInput").ap()
    consd_d = nc.dram_tensor("consd", (128, 256), F32R, kind="ExternalInput").ap()
    wprojT_d = nc.dram_tensor("wprojT", (DIM, DIM), F32R, kind="ExternalInput").ap()
    wp8hi_d = nc.dram_tensor("wp8hi", (128, 2, 2, DIM), F8,
                             kind="ExternalInput").ap()
    wp8lo_d = nc.dram_tensor("wp8lo", (128, 2, 2, DIM), F8,
                             kind="ExternalInput").ap()
    projb_d = nc.dram_tensor("projb6", (128, NC), F32, kind="ExternalInput").ap()
    idn_d = nc.dram_tensor("idn128", (128, 128), BF16, kind="ExternalInput").ap()
    y = nc.dram_tensor("y", (DIM, N), F32, kind="ExternalOutput").ap()

    with tile.TileContext(nc) as tc:
        # PE p-state warm-up under the initial DMA gate.
        if K_WARM:
            with tc.tile_pool(name="warm", bufs=1) as warm_p, \
                 tc.tile_pool(name="warmps", bufs=1, space="PSUM") as wps_p:
                jnk = warm_p.tile([128, 512], BF16)
                nc.vector.memset(jnk, 0.5)
                jps = wps_p.tile([128, 512], F32)
                for _ in range(K_WARM):
                    nc.tensor.matmul(jps, lhsT=jnk[:, 0:128], rhs=jnk,
                                     start=True, stop=True,
                                     skip_group_check=True)

        # ---- long-lived pools (bottom of SBUF stack) ----
        qall_p = tc.alloc_tile_pool(name="qall", bufs=1)
        kall_p = tc.alloc_tile_pool(name="kall", bufs=1)
        vall_p = tc.alloc_tile_pool(name="vall", bufs=1)
        cons_p = tc.alloc_tile_pool(name="cons", bufs=1)
        aod_p = tc.alloc_tile_pool(name="aod", bufs=1)
        w2a_p = tc.alloc_tile_pool(name="w2a", bufs=1)
        xt8_p = tc.alloc_tile_pool(name="xt8", bufs=1)
        w8_p = tc.alloc_tile_pool(name="w8p", bufs=1)

        # [par*64+r, slice, pair, token]; s0 = q/k features, s1 = bias rows
        QALL8 = qall_p.tile([128, 2, NPR, N], F8)
        KALL8 = kall_p.tile([128, 2, NPR, N], F8)
        VALL = vall_p.tile([128, NPR, NKB, VW], BF16)
        CONSR = cons_p.tile([128, 256], F32R)
        CONSB = cons_p.tile([1, 128], BF16)
        VBS = cons_p.tile([1, DIM], BF16)
        QKVB = cons_p.tile([128, 18], F32)
        AOD = aod_p.tile([128, NC, N], F32R)
        AOD8HI = aod_p.tile([128, 2, 2, N], F8)    # pairs c0..c3, DR layout
        AOD8LO = aod_p.tile([128, 2, 2, N], F8)
        PPART = aod_p.tile([128, NC, N], BF16)     # proj partial (c0..c3)
        WP = w2a_p.tile([128, NC, 2, 384], F32R)   # [:, c, g, :] proj cols
        W8PHI = w2a_p.tile([128, 2, 2, DIM], F8)
        W8PLO = w2a_p.tile([128, 2, 2, DIM], F8)
        PBIAS = w2a_p.tile([128, NC], F32)
        IDN128 = w2a_p.tile([128, 128], BF16)
        XT8HI = xt8_p.tile([128, NPAIR, 2, N], F8)
        XT8LO = xt8_p.tile([128, NPAIR, 2, N], F8)
        W8HI = w8_p.tile([128, NPAIR, 2, 3 * DIM], F8)
        W8LO = w8_p.tile([128, NPAIR, 2, 3 * DIM], F8)

        # ---- input DMAs, issue order = criticality ----
        nc.sync.dma_start(out=XT8HI.rearrange("p pr s t -> p (pr s) t"),
                          in_=x8hi_d)
        nc.sync.dma_start(out=W8HI[:, :, :, 0:DIM], in_=w8hi_d[:, :, :, 0:DIM])
        nc.sync.dma_start(out=XT8LO.rearrange("p pr s t -> p (pr s) t"),
                          in_=x8lo_d)
        nc.sync.dma_start(out=W8LO[:, :, :, 0:DIM], in_=w8lo_d[:, :, :, 0:DIM])
        nc.sync.dma_start(out=QKVB, in_=qkvb_d)

        nc.vector.memset(CONSB, 1.0)
        # V'' constant columns
        nc.vector.memset(VALL[:, :, :, 64:65], 1.0)
        nc.vector.memset(VALL[:, :, :, 65:97], 0.0)
        nc.vector.memset(VALL[:, :, :, 97:98], 1.0)
        nc.vector.memset(VALL[:, :, :, 98:129], 0.0)

        PASSES = ((0, 0), (0, 1), (1, 0))   # (w-lo?, x-lo?) per pass

        def emit_qkv_pass(ps, m, qh, pi):
            wt = (W8HI, W8LO)[PASSES[pi][0]]
            xt = (XT8HI, XT8LO)[PASSES[pi][1]]
            qsl = slice(qh * 512, (qh + 1) * 512)
            for p in range(NPAIR):
                nc.tensor.matmul(
                    ps, lhsT=wt[:, p, :, m * 128:(m + 1) * 128],
                    rhs=xt[:, p, :, qsl],
                    start=(pi == 0 and p == 0),
                    stop=(pi == 2 and p == NPAIR - 1), perf_mode=DR)

        def emit_qkv_drain(ps, m, qh):
            bias_t = QKVB[:, m:m + 1]
            qsl = slice(qh * 512, (qh + 1) * 512)
            if m < 6:
                nc.scalar.activation(QALL8[:, 0, m, qsl], ps, IDN,
                                     bias=bias_t, scale=1.0 / WS)
            else:
                nc.vector.tensor_scalar(
                    out=KALL8[:, 0, m - 6, qsl], in0=ps,
                    scalar1=1.0 / WS, scalar2=bias_t, op0=MULT, op1=ADD)

        def emit_qkv_half(m, qh, ps_pool, tag="ps1", bufs=None):
            """feature chunk m (q 0..5, k 6..11), query half qh; 3-pass DR."""
            ps = ps_pool.tile([128, 512], F32, tag=tag, bufs=bufs or K_PS1,
                              name=f"ps1_{m}_{qh}")
            for pi in range(3):
                emit_qkv_pass(ps, m, qh, pi)
            emit_qkv_drain(ps, m, qh)

        def emit_v_pair(pair, ps_pool, tb0, ntb, tag="psv", bufs=1):
            """v features for head pair; 2 token-blocks per psum group."""
            for g0 in range(tb0, tb0 + ntb, 2):
                psv = ps_pool.tile([128, 2, 128], F32, tag=tag, bufs=bufs,
                                   name=f"psv{pair}_{g0}")
                for j in range(2):
                    tb = g0 + j
                    tsl = slice(tb * 128, (tb + 1) * 128)
                    vsl = slice(2 * DIM + pair * 128, 2 * DIM + (pair + 1) * 128)
                    first = True
                    for wt, xt in ((W8HI, XT8HI), (W8HI, XT8LO), (W8LO, XT8HI)):
                        for p in range(NPAIR):
                            nc.tensor.matmul(
                                psv[:, j, :], lhsT=xt[:, p, :, tsl],
                                rhs=wt[:, p, :, vsl],
                                start=first, stop=False, perf_mode=DR)
                            first = False
                    nc.tensor.matmul(
                        psv[:, j, :], lhsT=CONSB,
                        rhs=VBS[:, pair * 128:(pair + 1) * 128],
                        start=False, stop=True)
                # drain both token blocks; parity-coded V'' destination
                vsrc = VALL[:, pair, g0, 0:64]
                vdst = bass.AP(tensor=vsrc.tensor, offset=vsrc.offset,
                               ap=[list(vsrc.ap[0]),
                                   [VW, 2], [129, 2], [1, 64]])
                psj = psv.rearrange("p tb (par h) -> p tb par h", h=64)
                nc.vector.tensor_copy(vdst, psj)

        # ---------- phase 1: q chunks, rel-pos, k chunk 0 ----------
        with tc.tile_pool(name="bd", bufs=1) as bd_p, \
             tc.tile_pool(name="ps1", bufs=1, space="PSUM") as ps1_p:
            BD = bd_p.tile([128, 2, 32, 128], F8)
            nc.sync.dma_start(out=BD, in_=bd8_d)
            # remaining weight/const DMAs (after critical path issues)
            nc.sync.dma_start(out=W8HI[:, :, :, DIM:2 * DIM],
                              in_=w8hi_d[:, :, :, DIM:2 * DIM])
            nc.sync.dma_start(out=W8LO[:, :, :, DIM:2 * DIM],
                              in_=w8lo_d[:, :, :, DIM:2 * DIM])
            nc.sync.dma_start(out=KALL8[:, 1, :, :], in_=kc8_d)
            nc.sync.dma_start(out=W8HI[:, :, :, 2 * DIM:],
                              in_=w8hi_d[:, :, :, 2 * DIM:])
            nc.sync.dma_start(out=W8LO[:, :, :, 2 * DIM:],
                              in_=w8lo_d[:, :, :, 2 * DIM:])
            nc.sync.dma_start(out=VBS, in_=vbrow_d)
            nc.sync.dma_start(out=CONSR, in_=consd_d)

            # rel-pos views: h copies are row-grouped, w copies col-grouped
            q8s0 = QALL8[:, 0]                                  # [128, 6, N]
            q8col = q8s0.rearrange("p c (t ww) -> p c t ww", ww=W)
            q8s1h = QALL8[:, 1].rearrange("p c (hb t) -> p hb c t", t=W)
            q8s1w = QALL8[:, 1].rearrange("p c (t wb) -> p wb c t", wb=W)
            RG = 4

            def emit_rel_h(i0, early):
                """h-values + explicit zeros in the w-partitions: ONE
                full-width copy per group (the bd stationary has zero
                columns outside its own partitions)."""
                ps_h = ps1_p.tile([128, RG, 256], F32, tag="ps2", bufs=K_PS2,
                                  name=f"psh{i0}")
                for j in range(RG):
                    h = i0 + j
                    nc.tensor.matmul(
                        ps_h[:, j, 0:192].rearrange("p (c t) -> p c t", t=32),
                        lhsT=BD[:, 0, h, :],
                        rhs=q8s0[:, :, h * 32:(h + 1) * 32],
                        start=True, stop=True, skip_group_check=True)
                rsl = slice(i0, i0 + RG)
                pr = ps_h[:, :, 0:192].rearrange("p hb (c t) -> p hb c t",
                                                 t=32)
                if early:
                    nc.vector.tensor_copy(q8s1h[:, rsl, :, :], pr)
                else:
                    nc.scalar.copy(q8s1h[:, rsl, :, :], pr)

            def emit_rel_w(i0, acc):
                """w-values: either a DVE full-width accumulate onto the
                h-copy zeros, or two narrow ACT copies of just the
                w-partitions (disjoint from the h rows)."""
                ps_w = ps1_p.tile([128, RG, 256], F32, tag="ps2", bufs=K_PS2,
                                  name=f"psw{i0}")
                for j in range(RG):
                    w = i0 + j
                    nc.tensor.matmul(
                        ps_w[:, j, 0:192].rearrange("p (c t) -> p c t", t=32),
                        lhsT=BD[:, 1, w, :],
                        rhs=q8col[:, :, :, w],
                        start=True, stop=True, skip_group_check=True)
                rsl = slice(i0, i0 + RG)
                pr = ps_w[:, :, 0:192].rearrange("p wb (c t) -> p wb c t",
                                                 t=32)
                if acc:
                    dst = q8s1w[:, rsl, :, :]
                    nc.vector.tensor_tensor(out=dst, in0=dst, in1=pr, op=ADD)
                else:
                    nc.scalar.copy(q8s1w[32:64, rsl, :, :], pr[32:64])
                    nc.scalar.copy(q8s1w[96:128, rsl, :, :], pr[96:128])

            # q chunks qh0; then qh1 interleaved with rel h-row groups that
            # only need qh0 tokens (rows 0..15); w matmuls need all tokens
            # and their accumulating copies must follow ALL h copies.
            for m in range(6):
                emit_qkv_half(m, 0, ps1_p)
            for m in range(6):
                emit_qkv_half(m, 1, ps1_p)
                if m >= 2:
                    emit_rel_h((m - 2) * RG, early=True)
            for i0 in range(16, H, RG):
                emit_rel_h(i0, early=False)
            # w wave, with k chunk 0 and V pair 0 in the PE slack
            emit_rel_w(0, True)
            emit_qkv_half(6, 0, ps1_p)
            emit_rel_w(4, False)
            emit_qkv_half(6, 1, ps1_p)
            emit_rel_w(8, True)
            emit_rel_w(12, False)
            emit_v_pair(0, ps1_p, 0, 2, tag='ps1', bufs=K_PS1)
            emit_rel_w(16, True)
            emit_v_pair(0, ps1_p, 2, 2, tag='ps1', bufs=K_PS1)
            emit_rel_w(20, False)
            emit_v_pair(0, ps1_p, 4, 2, tag='ps1', bufs=K_PS1)
            emit_rel_w(24, True)
            emit_v_pair(0, ps1_p, 6, 2, tag='ps1', bufs=K_PS1)
            emit_rel_w(28, True)

        # ---------- phase 2: attention (ACT-bound), fillers in PE slack ----
        with tc.tile_pool(name="pt", bufs=K_PT) as pt_p, \
             tc.tile_pool(name="sm", bufs=K_SM) as sm_p, \
             tc.tile_pool(name="pss", bufs=K_PSS, space="PSUM") as psS_p, \
             tc.tile_pool(name="pv", bufs=2, space="PSUM") as psPV_p, \
             tc.tile_pool(name="rb", bufs=1, space="PSUM") as psRB_p, \
             tc.tile_pool(name="psvp", bufs=1, space="PSUM") as psV_p:


            nc.sync.dma_start(
                out=WP[:, :, 0, :],
                in_=wprojT_d[:, 0:384].rearrange("(c p) f -> p c f", p=128))
            nc.sync.dma_start(
                out=WP[:, :, 1, :],
                in_=wprojT_d[:, 384:768].rearrange("(c p) f -> p c f", p=128))
            nc.sync.dma_start(out=PBIAS, in_=projb_d)
            def k_unit(c, qh):
                def run():
                    emit_qkv_half(6 + c, qh, psV_p, tag="psv", bufs=1)
                return run

            def v_unit(c, g):
                def run():
                    emit_v_pair(c, psV_p, 2 * g, 2)
                return run

            def proj_unit(idx, with_c4):
                ob, qh = idx % 6, idx // 6
                def run():
                    g, mi = ob // 3, ob % 3
                    qsl = slice(qh * 512, (qh + 1) * 512)
                    psE = psV_p.tile([128, 512], F32, tag="psv", bufs=1,
                                     name=f"pe{ob}_{qh}")
                    passes = ((W8PHI, AOD8HI), (W8PHI, AOD8LO),
                              (W8PLO, AOD8HI))
                    for pi, (whi, ahi) in enumerate(passes):
                        for rp in range(2):
                            last_dr = (pi == 2 and rp == 1)
                            nc.tensor.matmul(
                                psE, lhsT=whi[:, rp, :,
                                              ob * 128:(ob + 1) * 128],
                                rhs=ahi[:, rp, :, qsl],
                                start=(pi == 0 and rp == 0),
                                stop=(last_dr and not with_c4),
                                perf_mode=DR)
                    if with_c4:
                        nc.tensor.matmul(
                            psE, lhsT=WP[:, 4, g, mi * 128:(mi + 1) * 128],
                            rhs=AOD[:, 4, qsl], start=False, stop=True)
                    nc.vector.tensor_scalar(
                        out=PPART[:, ob, qsl], in0=psE,
                        scalar1=1.0 / WS, scalar2=PBIAS[:, ob:ob + 1],
                        op0=MULT, op1=ADD)
                return run

            kf_state = {}

            def filler(head, kb):
                c = head // 2 + 1
                if c > 5:
                    if head == 10 and kb == 2:
                        nc.sync.dma_start(
                            out=WP[:, :, 0, :],
                            in_=wprojT_d[:, 0:384].rearrange(
                                "(c p) f -> p c f", p=128))
                        nc.sync.dma_start(
                            out=WP[:, :, 1, :],
                            in_=wprojT_d[:, 384:768].rearrange(
                                "(c p) f -> p c f", p=128))
                        nc.sync.dma_start(out=PBIAS, in_=projb_d)
                    return
                if head % 2 == 0:
                    qh, pi = (0, kb - 2) if kb <= 4 else (1, kb - 5)
                    if pi == 0:
                        kf_state[qh] = psV_p.tile(
                            [128, 512], F32, tag="psv", bufs=1,
                            name=f"kf{c}_{qh}")
                    emit_qkv_pass(kf_state[qh], 6 + c, qh, pi)
                    if pi == 2:
                        emit_qkv_drain(kf_state[qh], 6 + c, qh)
                else:
                    g = {2: 0, 4: 1, 5: 2, 6: 3}.get(kb)
                    if g is not None:
                        emit_v_pair(c, psV_p, 2 * g, 2)

            def filler_flush():
                pass

            def emit_s_exp(head, kb):
                pair, par = head // 2, head % 2
                p0 = par * 64
                ps_s = psS_p.tile([128, 1024], F32, tag="pss")
                ksl = slice(kb * 128, (kb + 1) * 128)
                for qh in range(NQH):
                    nc.tensor.matmul(
                        ps_s[:, qh * 512:(qh + 1) * 512],
                        lhsT=KALL8[p0:p0 + 64, :, pair, ksl],
                        rhs=QALL8[p0:p0 + 64, :, pair,
                                  qh * 512:(qh + 1) * 512],
                        start=True, stop=True, perf_mode=DR)
                pt = pt_p.tile([128, 1024], BF16, tag="pt")
                nc.scalar.activation(pt, ps_s, EXP, scale=SCALE)
                return pt

            def emit_pv(head, kb, pv, pt):
                pair, par = head // 2, head % 2
                vsl = (slice(0, 65) if par == 0 else slice(65, 193))
                for qh in range(NQH):
                    pv_out = pv[qh][0:65] if par == 0 else pv[qh]
                    nc.tensor.matmul(
                        pv_out, lhsT=VALL[:, pair, kb, vsl],
                        rhs=pt[:, qh * 512:(qh + 1) * 512],
                        start=(kb == 0), stop=(kb == NKB - 1))

            def emit_norm(head, pv, last=False):
                pair, par = head // 2, head % 2
                dr_row = 64 if par == 0 else 32
                ao_rows = slice(0, 64) if par == 0 else slice(64, 128)
                for qh in range(NQH):
                    dsb = sm_p.tile([128, 512], F32R, tag="dsb",
                                    name=f"dsb{head}_{qh}")
                    if last:
                        nc.scalar.copy(dsb[dr_row:dr_row + 1],
                                       pv[qh][dr_row:dr_row + 1])
                    else:
                        nc.vector.tensor_copy(dsb[dr_row:dr_row + 1],
                                              pv[qh][dr_row:dr_row + 1])
                    rbt = psRB_p.tile([128, 512], F32, tag="rb",
                                      name=f"rb{head}_{qh}")
                    if par == 0:
                        nc.tensor.matmul(rbt[0:64], lhsT=CONSR[64:65, 0:64],
                                         rhs=dsb[64:65], start=True, stop=True)
                    else:
                        nc.tensor.matmul(rbt, lhsT=CONSR[32:33, 128:256],
                                         rhs=dsb[32:33], start=True, stop=True)
                    rbr = sm_p.tile([128, 512], F32, tag="rbr",
                                    name=f"rbr{head}_{qh}")
                    nc.vector.reciprocal(rbr[ao_rows], rbt[ao_rows])
                    qsl = slice(qh * 512, (qh + 1) * 512)
                    nc.vector.tensor_mul(
                        AOD[ao_rows, pair, qsl], pv[qh][ao_rows],
                        rbr[ao_rows])


            # software-pipelined two ways: every PV is emitted after the NEXT
            # step's S+exp (a stalled PV in the PE weight-load queue would
            # otherwise block the following S), and head h's norm is emitted
            # after head h+1's first S+exp so the exp chain never waits.
            prev_norm = None
            pending = None          # (head, kb, pv, pt) for the deferred PV
            pv = None
            for head in range(HEADS):
                for kb in range(NKB):
                    pt = emit_s_exp(head, kb)
                    if pending is not None:
                        emit_pv(*pending)
                    if kb == 0:
                        if prev_norm is not None:
                            emit_norm(prev_norm[0], prev_norm[1])
                        pv = [psPV_p.tile([128, 512], F32, tag="pv", bufs=2,
                                          name=f"pv{head}_{qh}")
                              for qh in range(NQH)]
                    pending = (head, kb, pv, pt)
                    if kb >= 2:
                        filler(head, kb)
                prev_norm = (head, pv)
            emit_pv(*pending)
            emit_norm(prev_norm[0], prev_norm[1], last=True)
            filler_flush()

        # ---------- phase 3: proj (fp32r) + 1/32 descale + bias ----------
        with tc.tile_pool(name="ysb", bufs=1) as ysb_p, \
             tc.tile_pool(name="ps4", bufs=K_PS4, space="PSUM") as ps4_p:
            YSB = ysb_p.tile([128, NC, N], F32)
            for ob in range(NC):
                for qh in range(NQH):
                    g, mi = ob // 3, ob % 3
                    ps = ps4_p.tile([128, 512], F32, tag="ps4",
                                    name=f"psp{qh}_{ob}")
                    qsl = slice(qh * 512, (qh + 1) * 512)
                    for c in range(NC):
                        nc.tensor.matmul(
                            ps, lhsT=WP[:, c, g, mi * 128:(mi + 1) * 128],
                            rhs=AOD[:, c, qsl],
                            start=(c == 0), stop=(c == NC - 1))
                    if (ob + qh) % 2 == 0:
                        nc.scalar.activation(YSB[:, ob, qsl], ps, IDN,
                                             bias=PBIAS[:, ob:ob + 1],
                                             scale=1.0 / WS)
                    else:
                        nc.vector.tensor_scalar(
                            out=YSB[:, ob, qsl], in0=ps, scalar1=1.0 / WS,
                            scalar2=PBIAS[:, ob:ob + 1], op0=MULT, op1=ADD)
                if ob in (1, 3):
                    nc.sync.dma_start(
                        out=y[(ob - 1) * 128:(ob + 1) * 128, :].rearrange(
                            "(c p) t -> p c t", p=128),
                        in_=YSB[:, ob - 1:ob + 1, :])
                elif ob >= 4:
                    nc.sync.dma_start(out=y[ob * 128:(ob + 1) * 128, :],
                                      in_=YSB[:, ob, :])

        w8_p.release()
        xt8_p.release()
        w2a_p.release()
        aod_p.release()
        cons_p.release()
        vall_p.release()
        kall_p.release()
        qall_p.release()

    nc.compile()
    return nc


def host_prep(x, qkv_w, qkv_b, proj_w, proj_b, rel_pos_h, rel_pos_w):
    """full inputs -> list of 8 per-core in_maps"""
    import ml_dtypes
    F8NP = ml_dtypes.float8_e4m3
    BFNP = ml_dtypes.bfloat16

    x = np.asarray(x, np.float32)
    qkv_w = np.asarray(qkv_w, np.float32)
    qkv_b = np.asarray(qkv_b, np.float32)
    proj_w = np.asarray(proj_w, np.float32)
    proj_b = np.asarray(proj_b, np.float32)
    rel_pos_h = np.asarray(rel_pos_h, np.float32)
    rel_pos_w = np.asarray(rel_pos_w, np.float32)

    # qkv weights: transpose, 32x scale, fp8 hi/lo, [128, pair, slice, col]
    w32 = np.ascontiguousarray(qkv_w.T) * WS            # [768, 2304]
    w8hi = w32.astype(F8NP)
    w8lo = (w32 - w8hi.astype(np.float32)).astype(F8NP)
    w8hi = np.ascontiguousarray(
        w8hi.reshape(NPAIR, 2, 128, 3 * DIM).transpose(2, 0, 1, 3))
    w8lo = np.ascontiguousarray(
        w8lo.reshape(NPAIR, 2, 128, 3 * DIM).transpose(2, 0, 1, 3))

    qkvb18 = np.ascontiguousarray(qkv_b.reshape(18, 128).T)

    # proj: feature-major [768, 768]; rows 0:512 also as 32x fp8 hi/lo pairs
    wprojT = np.ascontiguousarray(proj_w.T)
    projb6 = np.ascontiguousarray(proj_b.reshape(NC, 128).T)
    idn128 = np.eye(128, dtype=np.float32).astype(BFNP)
    wp32 = wprojT[0:512] * WS
    wp8hi = wp32.astype(F8NP)
    wp8lo = (wp32 - wp8hi.astype(np.float32)).astype(F8NP)
    wp8hi = np.ascontiguousarray(
        wp8hi.reshape(2, 2, 128, DIM).transpose(2, 0, 1, 3))
    wp8lo = np.ascontiguousarray(
        wp8lo.reshape(2, 2, 128, DIM).transpose(2, 0, 1, 3))

    # rel-pos block-diagonal tables, fp8, new column placement
    idx = np.arange(H)
    Rh = rel_pos_h[idx[:, None] - idx[None, :] + (H - 1)]  # (32,32,64)
    Rw = rel_pos_w[idx[:, None] - idx[None, :] + (W - 1)]
    bd8 = np.zeros((2, 32, 128, 128), np.float32)
    for h in range(H):
        bd8[0, h, 0:64, 0:32] = Rh[h].T / SCALE     # even heads -> part 0:32
        bd8[0, h, 64:128, 64:96] = Rh[h].T / SCALE  # odd heads -> part 64:96
    for w in range(W):
        bd8[1, w, 0:64, 32:64] = Rw[w].T / SCALE
        bd8[1, w, 64:128, 96:128] = Rw[w].T / SCALE
    bd8 = np.ascontiguousarray(bd8.transpose(2, 0, 1, 3)).astype(F8NP)

    # indicator rows, replicated for both parities and all 6 pairs
    k = np.arange(N)
    kconst = np.zeros((64, N), np.float32)
    kconst[:32] = (k[None, :] // 32 == np.arange(32)[:, None])
    kconst[32:] = (k[None, :] % 32 == np.arange(32)[:, None])
    kc2 = np.concatenate([kconst, kconst], axis=0)      # [128, N]
    kc8rep = np.ascontiguousarray(
        np.broadcast_to(kc2[:, None, :], (128, NPR, N))).astype(F8NP)

    consd = np.zeros((128, 256), np.float32)
    consd[:, 0:128] = 1.0
    consd[:, 192:256] = 1.0
    vbrow32 = (qkv_b[2 * DIM:] * WS).reshape(1, DIM).astype(BFNP)

    shared = dict(w8hi=w8hi, w8lo=w8lo, qkvb18=qkvb18, bd8=bd8,
                  kc8rep=kc8rep, vbrow32=vbrow32, consd=consd,
                  wprojT=wprojT, projb6=projb6, wp8hi=wp8hi, wp8lo=wp8lo,
                  idn128=idn128)
    in_maps = []
    for b in range(B):
        xT = np.ascontiguousarray(x[b].reshape(N, DIM).T)   # [768, 1024]
        x8hi = xT.astype(F8NP)
        x8lo = (xT - x8hi.astype(np.float32)).astype(F8NP)
        x8hi = np.ascontiguousarray(x8hi.reshape(NC, 128, N).transpose(1, 0, 2))
        x8lo = np.ascontiguousarray(x8lo.reshape(NC, 128, N).transpose(1, 0, 2))
        in_maps.append(dict(x8hi=x8hi, x8lo=x8lo, **shared))
    return in_maps


def get_nc():
    if "nc" not in _CACHE:
        _CACHE["nc"] = build_nc()
    return _CACHE["nc"]


def kernel(**inputs):
    nc = get_nc()
    in_maps = host_prep(**inputs)
    res = bass_utils.run_bass_kernel_spmd(nc, in_maps, core_ids=list(range(NCORES)))
    out = np.stack([np.asarray(r["y"]).T for r in res.results], axis=0)
    return np.ascontiguousarray(out).reshape(B, H, W, DIM).astype(np.float32)
